# revision 1
# baseline (speedup 1.0000x reference)
"""GAU (gated attention unit) Bass kernel for Trainium2, 8-core data-parallel.

Contract: kernel(**inputs) takes FULL inputs (x [8,2048,512] f32, W1, b1, W2,
b2, rope_a, rope_b, gamma, beta, norm_scale) and returns the full output
[8,2048,512] f32.  Internally: batch b -> NeuronCore b (all params
replicated), one fused Bass/Tile program per core, no collectives.

Math notes:
  - toeplitz bias T[m,n] = sum_j c_j cos((n-m)w_j) + s_j sin((n-m)w_j)
      c_j = a_j b_j + a_{h+j} b_{h+j},  s_j = a_{h+j} b_j - a_j b_{h+j}
    so T is generated by a length-4095 vector f(d).  We compute f (reversed,
    "frev") on device with two tiny matmuls against input-independent trig
    basis matrices (shipped from host), then add T tiles into the qk PSUM
    with a PE matmul against the anti-identity J: (J @ H)[p,e] = H[127-p,e]
    where H[p,e] = frev[B + p + e] is a plain positive-stride DMA load.
  - qk scale 1/2048 is folded into gamma[0]/beta[0].
"""

import os
from contextlib import ExitStack

import numpy as np
import ml_dtypes

import concourse.bass as bass
import concourse.bacc as bacc
import concourse.tile as tile
from concourse import mybir
from concourse.bass_utils import run_bass_kernel_spmd

F32 = mybir.dt.float32
BF16 = mybir.dt.bfloat16
AF = mybir.ActivationFunctionType
ALU = mybir.AluOpType

B, N, D = 8, 2048, 512
EXPAND, SHARED = 1024, 128
PROJ = 2 * EXPAND + SHARED  # 2176
EPS = 1e-6
HALF = N // 2  # 1024 rope freqs

NT = N // 128   # 16 seq tiles
KD = D // 128   # 4 contraction tiles for proj1
CU = EXPAND // 128  # 8 u channel tiles
MB = N // 512   # 4 m-blocks of 512

LAST_RESULTS = None  # test introspection


def _basis_matrices():
    """Input-independent trig bases for the on-device Frev computation.

    Frev[p,r] = f_idx(4095 - 64p - r) = f(2048 - 64p - r), built as
    CA @ G1 + SA @ G2 with G1 = CB*c + SB*s, G2 = CB*s - SB*c computed on
    device from the runtime rope_a/rope_b.
    """
    w = 10000.0 ** (-np.arange(HALF, dtype=np.float64) / HALF)
    p = np.arange(64)
    r = np.arange(64)
    alpha = np.outer(2048 - 64 * p, w)  # [64p, 1024j]
    beta = np.outer(-r, w)              # [64r, 1024j]
    ca = np.cos(alpha).T  # [1024j, 64p]  (lhsT: K=j, M=p)
    sa = np.sin(alpha).T
    cb = np.cos(beta).T   # [1024j, 64r]  (rhs template: K=j, N=r)
    sb = np.sin(beta).T
    bf = ml_dtypes.bfloat16
    return (ca.astype(bf), sa.astype(bf), cb.astype(bf), sb.astype(bf))


def _pack_smalls(rope_a, rope_b, gamma, beta, norm_scale):
    sm = np.zeros((128, 37), np.float32)
    sm[:, 0:16] = np.asarray(rope_a, np.float32).reshape(16, 128).T
    sm[:, 16:32] = np.asarray(rope_b, np.float32).reshape(16, 128).T
    g = np.asarray(gamma, np.float32)
    be = np.asarray(beta, np.float32)
    sm[:, 32] = g[0]
    sm[:, 33] = g[1]
    sm[:, 34] = be[0]
    sm[:, 35] = be[1]
    sm[:, 36] = np.float32(np.asarray(norm_scale).reshape(()))
    return sm


def _pack_basis():
    ca, sa, cb, sb = _basis_matrices()
    return np.concatenate([ca, sa, cb, sb], axis=1)  # [1024, 256] bf16


def _build(b1_zero: bool, b2_zero: bool, sim_compat: bool = False, stage: int = 2):
    nc = bacc.Bacc("TRN2", target_bir_lowering=False, debug=False)

    # ---- I/O ----
    x_d = nc.dram_tensor("x", [N, D], F32, kind="ExternalInput")
    w1_d = nc.dram_tensor("w1", [D, PROJ], BF16, kind="ExternalInput")
    w2_d = nc.dram_tensor("w2", [EXPAND, D], BF16, kind="ExternalInput")
    # smalls: [128, 37] f32, host-packed: 0:16 a[128t+p], 16:32 b[128t+p],
    # 32 gamma0, 33 gamma1, 34 beta0, 35 beta1 (as [128] cols), 36 norm_scale bcast
    sm_d = nc.dram_tensor("smalls", [128, 37], F32, kind="ExternalInput")
    # basis: [1024, 256] bf16: cols 0:64 ca, 64:128 sa, 128:192 cb, 192:256 sb
    bas_d = nc.dram_tensor("basis", [HALF, 256], BF16, kind="ExternalInput")
    b1_d = None if b1_zero else nc.dram_tensor("b1t", [128, 17], F32, kind="ExternalInput")
    b1b_d = None if b1_zero else nc.dram_tensor("b1bc", [128, EXPAND], F32, kind="ExternalInput")
    b2_d = None if b2_zero else nc.dram_tensor("b2bc", [128, D], F32, kind="ExternalInput")
    out_d = nc.dram_tensor("out", [N, D], F32, kind="ExternalOutput")

    frev_d = nc.dram_tensor("frev_scratch", [64, 64], BF16)  # internal

    with tile.TileContext(nc) as tc:
        with ExitStack() as ctx:
            _emit(ctx, tc, nc, locals(), sim_compat, stage)
    nc.compile()
    return nc


def _emit(ctx, tc, nc, t, sim_compat=False, stage=2):
    x_d, w1_d, w2_d = t["x_d"], t["w1_d"], t["w2_d"]
    sm_d, bas_d = t["sm_d"], t["bas_d"]
    b1_d, b1b_d = t["b1_d"], t["b1b_d"]
    b2_d, out_d, frev_d = t["b2_d"], t["out_d"], t["frev_d"]

    # ---------------- pools ----------------
    persist = ctx.enter_context(tc.tile_pool(name="persist", bufs=1))
    xpool = ctx.enter_context(tc.tile_pool(name="xpool", bufs=6))
    xnpool = ctx.enter_context(tc.tile_pool(name="xnpool", bufs=3))
    small = ctx.enter_context(tc.tile_pool(name="small", bufs=2))
    hpool = ctx.enter_context(tc.tile_pool(name="hpool", bufs=4))
    kernp = ctx.enter_context(tc.tile_pool(name="kernp", bufs=2))
    outst = ctx.enter_context(tc.tile_pool(name="outst", bufs=3))
    ps_a = ctx.enter_context(tc.tile_pool(name="ps_a", bufs=3, space="PSUM"))
    ps_o1 = ctx.enter_context(tc.tile_pool(name="ps_o1", bufs=4, space="PSUM"))


    silu_pool = ctx.enter_context(tc.tile_pool(name="silu_pool", bufs=3))

    def silu_evict(out_ap, ps_ap, bias=None, name="sig"):
        if not sim_compat:
            if bias is not None:
                nc.scalar.activation(out_ap, ps_ap, AF.Silu, bias=bias)
            else:
                nc.scalar.activation(out_ap, ps_ap, AF.Silu)
        else:
            assert bias is None
            sig = silu_pool.tile(list(ps_ap.shape), F32, tag="sig", name=name)
            nc.scalar.activation(sig[:], ps_ap, AF.Sigmoid)
            nc.vector.scalar_tensor_tensor(out=out_ap, in0=sig[:], scalar=1.0,
                                           in1=ps_ap, op0=ALU.mult, op1=ALU.mult)

    # ---------------- persistent SBUF ----------------
    W1sb = persist.tile([128, KD, PROJ], BF16)      # [d-part, kt, proj]
    W2sb = persist.tile([128, CU, D], BF16)         # [e-part, et, d]
    xnT = persist.tile([128, KD, N], BF16)          # [d-part, kt, m]
    uT = persist.tile([128, CU, N], BF16)           # [e-part, et, m]
    baseT = persist.tile([128, N], BF16)            # [s-part, m]
    qT = persist.tile([128, N], BF16)
    kT = persist.tile([128, N], BF16)
    vsb = persist.tile([128, NT, EXPAND], BF16)     # [n-part, nt, e]
    identB = persist.tile([128, 128], BF16)
    ident32 = persist.tile([128, 128], F32)
    jmatB = persist.tile([128, 128], BF16)

    for kt in range(KD):
        nc.sync.dma_start(out=W1sb[:, kt, :], in_=w1_d.ap()[128 * kt:128 * (kt + 1), :])
    for et in range(CU):
        nc.sync.dma_start(out=W2sb[:, et, :], in_=w2_d.ap()[128 * et:128 * (et + 1), :])

    # identity / anti-identity (bf16) for PE transpose + toeplitz row-flip
    nc.gpsimd.memset(identB[:], 0.0)
    nc.gpsimd.affine_select(out=identB[:], in_=identB[:], compare_op=ALU.not_equal,
                            fill=1.0, base=0, pattern=[[-1, 128]], channel_multiplier=1)
    nc.gpsimd.memset(ident32[:], 0.0)
    nc.gpsimd.affine_select(out=ident32[:], in_=ident32[:], compare_op=ALU.not_equal,
                            fill=1.0, base=0, pattern=[[-1, 128]], channel_multiplier=1)
    nc.gpsimd.memset(jmatB[:], 0.0)
    nc.gpsimd.affine_select(out=jmatB[:], in_=jmatB[:], compare_op=ALU.not_equal,
                            fill=1.0, base=-127, pattern=[[1, 128]], channel_multiplier=1)

    # ---------------- smalls + rope -> frev (one-time, tiny) ----------------
    do_frev = stage >= 1
    smalls = persist.tile([128, 37], F32)
    nc.sync.dma_start(out=smalls[:], in_=sm_d.ap()[:, :])
    cs = persist.tile([128, 2, 8], F32)  # [:,0,:]=c, [:,1,:]=s
    tmp8a = persist.tile([128, 8], F32)
    tmp8b = persist.tile([128, 8], F32)
    # stage smalls into a DVE-produced copy so later DVE ops have all-DVE deps
    smv = persist.tile([128, 37], F32)
    nc.vector.tensor_copy(smv[:], smalls[:])
    asl, bsl2 = smv[:, 0:16], smv[:, 16:32]
    # c = a1*b1 + a2*b2
    nc.vector.tensor_tensor(out=tmp8a[:], in0=asl[:, 0:8], in1=bsl2[:, 0:8], op=ALU.mult)
    nc.vector.scalar_tensor_tensor(out=cs[:, 0, :], in0=asl[:, 8:16], scalar=1.0,
                                   in1=bsl2[:, 8:16], op0=ALU.mult, op1=ALU.mult)
    nc.vector.tensor_tensor(out=cs[:, 0, :], in0=cs[:, 0, :], in1=tmp8a[:], op=ALU.add)
    # s = a2*b1 - a1*b2
    nc.vector.tensor_tensor(out=tmp8b[:], in0=asl[:, 0:8], in1=bsl2[:, 8:16], op=ALU.mult)
    nc.vector.scalar_tensor_tensor(out=cs[:, 1, :], in0=asl[:, 8:16], scalar=1.0,
                                   in1=bsl2[:, 0:8], op0=ALU.mult, op1=ALU.mult)
    nc.vector.tensor_tensor(out=cs[:, 1, :], in0=cs[:, 1, :], in1=tmp8b[:], op=ALU.subtract)

    if do_frev:
        BAS = persist.tile([128, 8, 256], BF16)  # [j-part, jt, ca|sa|cb|sb]
        for jt in range(8):
            nc.sync.dma_start(out=BAS[:, jt, :], in_=bas_d.ap()[128 * jt:128 * (jt + 1), :])
        G1 = persist.tile([128, 8, 64], BF16)
        G2 = persist.tile([128, 8, 64], BF16)
        CSst = persist.tile([128, 8, 128], BF16)  # staged cb|sb per jt (DVE-produced)
        for jt in range(8):
            nc.vector.tensor_copy(CSst[:, jt, :], BAS[:, jt, 128:256])
        for jt in range(8):
            ccol = cs[:, 0, jt:jt + 1]
            scol = cs[:, 1, jt:jt + 1]
            cbj, sbj = CSst[:, jt, 0:64], CSst[:, jt, 64:128]
            tmp = small.tile([128, 64], F32, tag="gtmp", name=f"gtmp_{jt}")
            nc.vector.tensor_scalar(out=tmp[:], in0=sbj, scalar1=scol, scalar2=None, op0=ALU.mult)
            nc.vector.scalar_tensor_tensor(out=G1[:, jt, :], in0=cbj, scalar=ccol,
                                           in1=tmp[:], op0=ALU.mult, op1=ALU.add)
            tmp2 = small.tile([128, 64], F32, tag="gtmp2", name=f"gtmp2_{jt}")
            nc.vector.tensor_scalar(out=tmp2[:], in0=sbj, scalar1=ccol, scalar2=None, op0=ALU.mult)
            nc.vector.scalar_tensor_tensor(out=G2[:, jt, :], in0=cbj, scalar=scol,
                                           in1=tmp2[:], op0=ALU.mult, op1=ALU.subtract)
        ps_f = ps_a.tile([64, 64], F32, tag="ps_a")
        for jt in range(8):
            nc.tensor.matmul(out=ps_f[:], lhsT=BAS[:, jt, 0:64], rhs=G1[:, jt, :],
                             start=(jt == 0), stop=False)
            nc.tensor.matmul(out=ps_f[:], lhsT=BAS[:, jt, 64:128], rhs=G2[:, jt, :],
                             start=False, stop=(jt == 7))
        frev_sb = small.tile([64, 64], BF16, tag="frev")
        nc.vector.tensor_copy(frev_sb[:], ps_f[:])
        nc.sync.dma_start(out=frev_d.ap()[:, :], in_=frev_sb[:])


    # ---------------- gamma/beta (q-side scaled by 1/N on device) ----------------
    gb = persist.tile([128, 2], F32)  # g0/N, b0/N
    nc.vector.tensor_scalar(out=gb[:, 0:1], in0=smv[:, 32:33], scalar1=1.0 / N, scalar2=None, op0=ALU.mult)
    nc.vector.tensor_scalar(out=gb[:, 1:2], in0=smv[:, 34:35], scalar1=1.0 / N, scalar2=None, op0=ALU.mult)

    b1col = None
    b1bc = None
    if b1_d is not None:
        b1col = persist.tile([128, 17], F32)
        nc.sync.dma_start(out=b1col[:], in_=b1_d.ap()[:, :])
        b1bc = persist.tile([128, EXPAND], F32)
        nc.sync.dma_start(out=b1bc[:], in_=b1b_d.ap()[:, :])
    b2bc = None
    if b2_d is not None:
        b2bc = persist.tile([128, D], F32)
        nc.sync.dma_start(out=b2bc[:], in_=b2_d.ap()[:, :])

    # ---------------- norm + xn + transpose ----------------
    mscol = persist.tile([128, NT], F32)
    rcol = persist.tile([128, NT], F32)
    sqscr = persist.tile([128, D], F32)
    for g in range(NT // 4):  # groups of 4 bound x-tile liveness
        xg = []
        for k4 in range(4):
            mt = 4 * g + k4
            x_t = xpool.tile([128, D], F32, tag="x", name=f"x_{mt}")
            nc.sync.dma_start(out=x_t[:], in_=x_d.ap()[128 * mt:128 * (mt + 1), :])
            nc.scalar.activation(sqscr[:], x_t[:], AF.Square,
                                 accum_out=mscol[:, mt:mt + 1])
            xg.append(x_t)
        gsl = slice(4 * g, 4 * g + 4)
        # rs = norm_scale / sqrt(ms/D + eps)
        nc.vector.tensor_scalar(out=rcol[:, gsl], in0=mscol[:, gsl], scalar1=1.0 / D,
                                scalar2=EPS, op0=ALU.mult, op1=ALU.add)
        nc.scalar.sqrt(rcol[:, gsl], rcol[:, gsl])
        nc.vector.reciprocal(rcol[:, gsl], rcol[:, gsl])
        nc.vector.tensor_scalar(out=rcol[:, gsl], in0=rcol[:, gsl], scalar1=smv[:, 36:37],
                                scalar2=None, op0=ALU.mult)
        for k4 in range(4):
            mt = 4 * g + k4
            xn_t = xnpool.tile([128, D], BF16, tag="xn", name=f"xn_{mt}")
            nc.vector.tensor_scalar(out=xn_t[:], in0=xg[k4][:], scalar1=rcol[:, mt:mt + 1],
                                    scalar2=None, op0=ALU.mult)
            for dt in range(KD):
                ps_tr = ps_a.tile([128, 1024], BF16, tag="ps_a", name=f"ps_tr_{mt}_{dt}")
                nc.tensor.transpose(out=ps_tr[:, 0:128], in_=xn_t[:, 128 * dt:128 * (dt + 1)],
                                    identity=identB[:])
                nc.vector.tensor_copy(xnT[:, dt, 128 * mt:128 * (mt + 1)], ps_tr[:, 0:128])

    if stage <= 0:
        for mt in range(NT):
            o_t = outst.tile([128, D], F32, tag="out", name=f"dbg0_{mt}")
            nc.scalar.activation(o_t[:], xnT[:, mt % KD, 512:1024], AF.Copy)
            nc.sync.dma_start(out=out_d.ap()[128 * mt:128 * (mt + 1), :], in_=o_t[:])
        return

    # ---------------- proj1: uT, baseT (transposed), v (natural) ----------------
    # uT[c, m] (stationary W1), silu applied at PSUM evict
    for cu in range(CU):
        csl = slice(128 * cu, 128 * (cu + 1))
        for mb in range(MB):
            msl = slice(512 * mb, 512 * (mb + 1))
            ps = ps_a.tile([128, 512], F32, tag="ps_a")
            for kt in range(KD):
                nc.tensor.matmul(out=ps[:], lhsT=W1sb[:, kt, csl], rhs=xnT[:, kt, msl],
                                 start=(kt == 0), stop=(kt == KD - 1))
            silu_evict(uT[:, cu, msl], ps[:],
                       bias=None if b1col is None else b1col[:, cu:cu + 1],
                       name=f"sig_u_{cu}_{mb}")
    # baseT [s, m]
    bsl = slice(2 * EXPAND, 2 * EXPAND + SHARED)
    for mb in range(MB):
        msl = slice(512 * mb, 512 * (mb + 1))
        ps = ps_a.tile([128, 512], F32, tag="ps_a")
        for kt in range(KD):
            nc.tensor.matmul(out=ps[:], lhsT=W1sb[:, kt, bsl], rhs=xnT[:, kt, msl],
                             start=(kt == 0), stop=(kt == KD - 1))
        silu_evict(baseT[:, msl], ps[:],
                   bias=None if b1col is None else b1col[:, 16:17],
                   name=f"sig_b_{mb}")
    # v [n, e] (stationary xnT)
    for nt in range(NT):
        nsl = slice(128 * nt, 128 * (nt + 1))
        for vh in range(2):
            vsl = slice(EXPAND + 512 * vh, EXPAND + 512 * (vh + 1))
            ps = ps_a.tile([128, 512], F32, tag="ps_a")
            for kt in range(KD):
                nc.tensor.matmul(out=ps[:], lhsT=xnT[:, kt, nsl], rhs=W1sb[:, kt, vsl],
                                 start=(kt == 0), stop=(kt == KD - 1))
            if b1bc is not None:
                zb = silu_pool.tile([128, 512], F32, tag="zb", name=f"zb_{nt}_{vh}")
                nc.vector.tensor_tensor(out=zb[:], in0=ps[:], in1=b1bc[:, 512 * vh:512 * (vh + 1)], op=ALU.add)
                silu_evict(vsb[:, nt, 512 * vh:512 * (vh + 1)], zb[:], name=f"sig_v_{nt}_{vh}")
            else:
                silu_evict(vsb[:, nt, 512 * vh:512 * (vh + 1)], ps[:], name=f"sig_v_{nt}_{vh}")

    if stage <= 1:
        for mt in range(NT):
            o_t = outst.tile([128, D], F32, tag="out", name=f"dbg_{mt}")
            nc.scalar.activation(o_t[:], uT[:, mt % CU, 512:1024], AF.Copy)
            nc.sync.dma_start(out=out_d.ap()[128 * mt:128 * (mt + 1), :], in_=o_t[:])
        return

    # q/k from baseT (scale 1/N folded into g0/b0)
    nc.vector.tensor_scalar(out=qT[:], in0=baseT[:], scalar1=gb[:, 0:1], scalar2=gb[:, 1:2],
                            op0=ALU.mult, op1=ALU.add)
    nc.vector.tensor_scalar(out=kT[:], in0=baseT[:], scalar1=smv[:, 33:34], scalar2=smv[:, 35:36],
                            op0=ALU.mult, op1=ALU.add)

    o_sb = persist.tile([128, CU, 512], BF16)

    # ---------------- attention + output proj, per m-block ----------------
    for mb in range(MB):
        msl = slice(512 * mb, 512 * (mb + 1))
        kernT = kernp.tile([128, NT, 512], BF16, tag="kernT", name=f"kernT_{mb}")
        for i in range(NT):
            ps = ps_a.tile([128, 512], F32, tag="ps_a")
            nc.tensor.matmul(out=ps[:], lhsT=kT[:, 128 * i:128 * (i + 1)], rhs=qT[:, msl],
                             start=True, stop=False)
            dd = i - 4 * mb
            base_off = 1921 - 128 * dd
            h_t = hpool.tile([128, 512], BF16, tag="h")
            nc.sync.dma_start(out=h_t[:], in_=bass.AP(tensor=frev_d, offset=base_off,
                                                      ap=[[1, 128], [1, 512]]))
            nc.tensor.matmul(out=ps[:], lhsT=jmatB[:], rhs=h_t[:], start=False, stop=True)
            # sqrrelu: relu on ACT (PSUM read), square on DVE (bf16 SBUF 2x)
            r_t = hpool.tile([128, 512], BF16, tag="rl", name=f"rl_{mb}_{i}")
            nc.scalar.activation(r_t[:], ps[:], AF.Relu)
            nc.vector.tensor_tensor(out=kernT[:, i, :], in0=r_t[:], in1=r_t[:], op=ALU.mult)
        pass
        for ep in range(2):
            pso = [ps_o1.tile([128, 512], F32, tag="o1", name=f"o1_{mb}_{ep}_{j}") for j in range(4)]
            for i in range(NT):
                for et4 in range(4):
                    et = 4 * ep + et4
                    nc.tensor.matmul(out=pso[et4][:], lhsT=vsb[:, i, 128 * et:128 * (et + 1)],
                                     rhs=kernT[:, i, :], start=(i == 0), stop=(i == NT - 1))
            for et4 in range(4):
                et = 4 * ep + et4
                # o = u * o1
                nc.vector.scalar_tensor_tensor(out=o_sb[:, et, :], in0=pso[et4][:], scalar=1.0,
                                               in1=uT[:, et, msl], op0=ALU.mult, op1=ALU.mult)
        # output projection for this m-block; +x residual (and +b2) folded into
        # the PSUM accumulation via identity / ones-row matmuls (keeps the DVE
        # out of the multi-producer join: DVE ops only get 1 sync-wait slot)
        for mt4 in range(4):
            mrow = 512 * mb + 128 * mt4
            ps = ps_a.tile([128, 512], F32, tag="ps_a")
            for et in range(CU):
                nc.tensor.matmul(out=ps[:], lhsT=o_sb[:, et, 128 * mt4:128 * (mt4 + 1)],
                                 rhs=W2sb[:, et, :], start=(et == 0), stop=False)
            x_t = xpool.tile([128, D], F32, tag="x", name=f"xr_{mb}_{mt4}")
            nc.sync.dma_start(out=x_t[:], in_=x_d.ap()[mrow:mrow + 128, :])
            if b2bc is not None:
                nc.tensor.matmul(out=ps[:], lhsT=ident32[:, 0:1].to_broadcast([1, 128]),
                                 rhs=b2bc[0:1, :], start=False, stop=False)
            nc.tensor.matmul(out=ps[:], lhsT=ident32[:], rhs=x_t[:], start=False, stop=True)
            o_t = outst.tile([128, D], F32, tag="out", name=f"ot_{mb}_{mt4}")
            nc.scalar.activation(o_t[:], ps[:], AF.Copy)
            nc.sync.dma_start(out=out_d.ap()[mrow:mrow + 128, :], in_=o_t[:])


_BUILD_CACHE = {}


def _get_nc(b1_zero, b2_zero, sim_compat=False, stage=2):
    key = (b1_zero, b2_zero, sim_compat, stage)
    if key not in _BUILD_CACHE:
        _BUILD_CACHE[key] = _build(b1_zero, b2_zero, sim_compat, stage)
    return _BUILD_CACHE[key]


def kernel(x, W1, b1, W2, b2, rope_a, rope_b, gamma, beta, norm_scale):
    global LAST_RESULTS
    x = np.asarray(x, dtype=np.float32)
    bf = ml_dtypes.bfloat16
    b1_zero = not np.any(np.asarray(b1))
    b2_zero = not np.any(np.asarray(b2))
    nc = _get_nc(b1_zero, b2_zero, stage=int(os.environ.get('GAU_STAGE', '2')))

    common = {
        "w1": np.asarray(W1, np.float32).astype(bf),
        "w2": np.asarray(W2, np.float32).astype(bf),
        "smalls": _pack_smalls(rope_a, rope_b, gamma, beta, norm_scale),
        "basis": _pack_basis(),
    }
    if not b1_zero:
        b1f = np.asarray(b1, np.float32)
        common["b1t"] = np.ascontiguousarray(b1f.reshape(17, 128).T)
        common["b1bc"] = np.broadcast_to(b1f[EXPAND:2 * EXPAND], (128, EXPAND)).copy()
    if not b2_zero:
        common["b2bc"] = np.broadcast_to(np.asarray(b2, np.float32), (128, D)).copy()

    in_maps = [dict(common, x=np.ascontiguousarray(x[i])) for i in range(B)]
    res = run_bass_kernel_spmd(nc, in_maps, list(range(B)),
                               trace=bool(os.environ.get("GAU_TRACE")))
    LAST_RESULTS = res
    out = np.stack([res.results[i]["out"] for i in range(B)]).astype(np.float32)
    return out



# revision 13
# speedup vs baseline: 1.6894x; 1.6894x over previous
"""GAU (gated attention unit) Bass kernel for Trainium2, 8-core data-parallel.

Contract: kernel(**inputs) takes FULL inputs (x [8,2048,512] f32, W1, b1, W2,
b2, rope_a, rope_b, gamma, beta, norm_scale) and returns the full output
[8,2048,512] f32.  Internally: batch b -> NeuronCore b (all params
replicated), one fused Bass/Tile program per core, no collectives.

Math notes:
  - toeplitz bias T[m,n] = sum_j c_j cos((n-m)w_j) + s_j sin((n-m)w_j)
      c_j = a_j b_j + a_{h+j} b_{h+j},  s_j = a_{h+j} b_j - a_j b_{h+j}
    so T is generated by a length-4095 vector f(d).  We compute f (reversed,
    "frev") on device with two tiny matmuls against input-independent trig
    basis matrices (shipped from host), then add T tiles into the qk PSUM
    with a PE matmul against the anti-identity J: (J @ H)[p,e] = H[127-p,e]
    where H[p,e] = frev[B + p + e] is a plain positive-stride DMA load.
    The 28 distinct diagonal-block H tiles are cached in SBUF.
  - fp8 (e4m3, +-240) DoubleRow matmuls for proj1, kernel@v and the output
    projection.  Static power-of-2 scales keep everything in fp8 range:
      W1,W2 x1024 (host)   xn x16   q-side x128/N   frev x128   v x64
    so  uv_psum = 2^14 uv   qk_psum = 128 (qk+T)   kern = relu(z)z = 2^14 kern
        o1_psum = 2^20 o1   o_sb = 2^8 o (fp8)     out_psum = 2^18 o@W2.
  - relu^2 is one DVE op per tile: kern = (ps max 0) * ps -> fp8.
  - residual is a DVE add at the final PSUM evict (x kept resident in SBUF).
"""

import os
from contextlib import ExitStack

import numpy as np
import ml_dtypes

import concourse.bass as bass
import concourse.bacc as bacc
import concourse.tile as tile
from concourse import mybir
from concourse.bass_utils import run_bass_kernel_spmd

F32 = mybir.dt.float32
BF16 = mybir.dt.bfloat16
F8 = mybir.dt.float8e4
AF = mybir.ActivationFunctionType
ALU = mybir.AluOpType
DR = mybir.MatmulPerfMode.DoubleRow

B, N, D = 8, 2048, 512
EXPAND, SHARED = 1024, 128
PROJ = 2 * EXPAND + SHARED  # 2176
EPS = 1e-6
HALF = N // 2  # 1024 rope freqs

NT = N // 128   # 16 seq tiles
KD = D // 128   # 4 contraction tiles for proj1
CU = EXPAND // 128  # 8 u channel tiles
MB = N // 512   # 4 m-blocks of 512
NDD = NT - 1 + MB * 4 - 3  # 28 distinct diagonal blocks (dd in [-12, 15])

S_W = 1024.0        # host scale on W1 and W2
S_X = 16.0          # xn scale (folded into rcol)
INV_UV = 1.0 / (S_W * S_X)   # 2^-14, unfolds uv psum at silu evict
S_QK = 128.0        # qk psum scale (q-side gamma + frev)
S_V = 1.0           # v written fp8 directly by ACT silu
OSCALE = 2.0 ** -6   # = S_o / (S_QK^2 * S_V) with S_o = 256
FSCALE = 2.0 ** -18  # = 1 / (S_o * S_W)

LAST_RESULTS = None  # test introspection


def _basis_matrices():
    """Input-independent trig bases for the on-device Frev computation.

    Frev[p,r] = f_idx(4095 - 64p - r) = f(2048 - 64p - r), built as
    CA @ G1 + SA @ G2 with G1 = CB*c + SB*s, G2 = CB*s - SB*c computed on
    device from the runtime rope_a/rope_b.
    """
    w = 10000.0 ** (-np.arange(HALF, dtype=np.float64) / HALF)
    p = np.arange(64)
    r = np.arange(64)
    alpha = np.outer(2048 - 64 * p, w)  # [64p, 1024j]
    beta = np.outer(-r, w)              # [64r, 1024j]
    ca = np.cos(alpha).T  # [1024j, 64p]  (lhsT: K=j, M=p)
    sa = np.sin(alpha).T
    cb = np.cos(beta).T   # [1024j, 64r]  (rhs template: K=j, N=r)
    sb = np.sin(beta).T
    bf = ml_dtypes.bfloat16
    return (ca.astype(bf), sa.astype(bf), cb.astype(bf), sb.astype(bf))


def _pack_smalls(rope_a, rope_b, gamma, beta, norm_scale):
    sm = np.zeros((128, 37), np.float32)
    sm[:, 0:16] = np.asarray(rope_a, np.float32).reshape(16, 128).T
    sm[:, 16:32] = np.asarray(rope_b, np.float32).reshape(16, 128).T
    g = np.asarray(gamma, np.float32)
    be = np.asarray(beta, np.float32)
    sm[:, 32] = g[0]
    sm[:, 33] = g[1]
    sm[:, 34] = be[0]
    sm[:, 35] = be[1]
    sm[:, 36] = np.float32(np.asarray(norm_scale).reshape(()))
    return sm


def _pack_basis():
    ca, sa, cb, sb = _basis_matrices()
    return np.concatenate([ca, sa, cb, sb], axis=1)  # [1024, 256] bf16


def _build(b1_zero: bool, b2_zero: bool, sim_compat: bool = False, stage: int = 2):
    nc = bacc.Bacc("TRN2", target_bir_lowering=False, debug=False)

    # ---- I/O ----
    x_d = nc.dram_tensor("x", [N, D], F32, kind="ExternalInput")
    w1_d = nc.dram_tensor("w1", [D, PROJ], F8, kind="ExternalInput")
    w2_d = nc.dram_tensor("w2", [EXPAND, D], F8, kind="ExternalInput")
    # smalls: [128, 37] f32, host-packed: 0:16 a[128t+p], 16:32 b[128t+p],
    # 32 gamma0, 33 gamma1, 34 beta0, 35 beta1 (as [128] cols), 36 norm_scale bcast
    sm_d = nc.dram_tensor("smalls", [128, 37], F32, kind="ExternalInput")
    # basis: [1024, 256] bf16: cols 0:64 ca, 64:128 sa, 128:192 cb, 192:256 sb
    bas_d = nc.dram_tensor("basis", [HALF, 256], BF16, kind="ExternalInput")
    b1_d = None if b1_zero else nc.dram_tensor("b1t", [128, 17], F32, kind="ExternalInput")
    b1b_d = None if b1_zero else nc.dram_tensor("b1bc", [128, EXPAND], F32, kind="ExternalInput")
    b2_d = None if b2_zero else nc.dram_tensor("b2bc", [128, D], F32, kind="ExternalInput")
    out_d = nc.dram_tensor("out", [N, D], F32, kind="ExternalOutput")

    frev_d = nc.dram_tensor("frev_scratch", [64, 64], BF16)  # internal

    with tile.TileContext(nc) as tc:
        with ExitStack() as ctx:
            _emit(ctx, tc, nc, locals(), sim_compat, stage)
    nc.compile()
    return nc


def _emit(ctx, tc, nc, t, sim_compat=False, stage=2):
    x_d, w1_d, w2_d = t["x_d"], t["w1_d"], t["w2_d"]
    sm_d, bas_d = t["sm_d"], t["bas_d"]
    b1_d, b1b_d = t["b1_d"], t["b1b_d"]
    b2_d, out_d, frev_d = t["b2_d"], t["out_d"], t["frev_d"]

    # ---------------- pools ----------------
    persist = ctx.enter_context(tc.tile_pool(name="persist", bufs=1))
    xnpool = ctx.enter_context(tc.tile_pool(name="xnpool", bufs=3))
    small = ctx.enter_context(tc.tile_pool(name="small", bufs=2))
    kernp = ctx.enter_context(tc.tile_pool(name="kernp", bufs=4))
    rlp = ctx.enter_context(tc.tile_pool(name="rlp", bufs=6))
    osbp = ctx.enter_context(tc.tile_pool(name="osbp", bufs=2))
    outst = ctx.enter_context(tc.tile_pool(name="outst", bufs=3))
    ps_a = ctx.enter_context(tc.tile_pool(name="ps_a", bufs=4, space="PSUM"))
    ps_o1 = ctx.enter_context(tc.tile_pool(name="ps_o1", bufs=4, space="PSUM"))

    silu_pool = ctx.enter_context(tc.tile_pool(name="silu_pool", bufs=3))

    def silu_evict(out_ap, ps_ap, bias=None, name="sig"):
        # out = silu(ps * INV_UV + bias)
        if bias is not None:
            nc.scalar.activation(out_ap, ps_ap, AF.Silu, bias=bias, scale=INV_UV)
        else:
            nc.scalar.activation(out_ap, ps_ap, AF.Silu, scale=INV_UV)

    # ---------------- persistent SBUF ----------------
    W1sb = persist.tile([128, KD, PROJ], F8)        # [d-part, kt, proj]
    W2sb = persist.tile([128, CU, D], F8)           # [e-part, et, d]
    xnT = persist.tile([128, KD, N], F8)            # [d-part, kt, m] (x16)
    uT = persist.tile([128, CU, N], BF16)           # [e-part, et, m]
    baseT = persist.tile([128, N], BF16)            # [s-part, m]
    qT = persist.tile([128, N], BF16)
    kT = persist.tile([128, N], BF16)
    vsb = persist.tile([128, NT, EXPAND], F8)       # [n-part, nt, e] (x64)
    xsb = persist.tile([128, NT, D], F32)           # resident x (norm + residual)
    Hcache = persist.tile([128, NDD, 512], BF16)    # toeplitz H per diagonal dd
    identB = persist.tile([128, 128], BF16)
    jmatB = persist.tile([128, 128], BF16)

    # x tiles 0-3 first (they gate the norm -> xnT -> proj1 chain), then W1
    for mt in range(4):
        nc.sync.dma_start(out=xsb[:, mt, :], in_=x_d.ap()[128 * mt:128 * (mt + 1), :])
    for kt in range(KD):
        nc.sync.dma_start(out=W1sb[:, kt, :], in_=w1_d.ap()[128 * kt:128 * (kt + 1), :])
    for mt in range(4, NT):
        nc.sync.dma_start(out=xsb[:, mt, :], in_=x_d.ap()[128 * mt:128 * (mt + 1), :])

    # identity / anti-identity (bf16) for PE transpose + toeplitz row-flip
    nc.gpsimd.memset(identB[:], 0.0)
    nc.gpsimd.affine_select(out=identB[:], in_=identB[:], compare_op=ALU.not_equal,
                            fill=1.0, base=0, pattern=[[-1, 128]], channel_multiplier=1)
    nc.gpsimd.memset(jmatB[:], 0.0)
    nc.gpsimd.affine_select(out=jmatB[:], in_=jmatB[:], compare_op=ALU.not_equal,
                            fill=1.0, base=-127, pattern=[[1, 128]], channel_multiplier=1)

    # ---------------- smalls + rope -> frev (one-time, tiny) ----------------
    do_frev = stage >= 1
    smalls = persist.tile([128, 37], F32)
    nc.sync.dma_start(out=smalls[:], in_=sm_d.ap()[:, :])
    cs = persist.tile([128, 2, 8], F32)  # [:,0,:]=c, [:,1,:]=s
    tmp8a = persist.tile([128, 8], F32)
    tmp8b = persist.tile([128, 8], F32)
    # stage smalls into a DVE-produced copy so later DVE ops have all-DVE deps
    smv = persist.tile([128, 37], F32)
    nc.vector.tensor_copy(smv[:], smalls[:])
    asl, bsl2 = smv[:, 0:16], smv[:, 16:32]
    # c = a1*b1 + a2*b2
    nc.vector.tensor_tensor(out=tmp8a[:], in0=asl[:, 0:8], in1=bsl2[:, 0:8], op=ALU.mult)
    nc.vector.scalar_tensor_tensor(out=cs[:, 0, :], in0=asl[:, 8:16], scalar=1.0,
                                   in1=bsl2[:, 8:16], op0=ALU.mult, op1=ALU.mult)
    nc.vector.tensor_tensor(out=cs[:, 0, :], in0=cs[:, 0, :], in1=tmp8a[:], op=ALU.add)
    # s = a2*b1 - a1*b2
    nc.vector.tensor_tensor(out=tmp8b[:], in0=asl[:, 0:8], in1=bsl2[:, 8:16], op=ALU.mult)
    nc.vector.scalar_tensor_tensor(out=cs[:, 1, :], in0=asl[:, 8:16], scalar=1.0,
                                   in1=bsl2[:, 0:8], op0=ALU.mult, op1=ALU.mult)
    nc.vector.tensor_tensor(out=cs[:, 1, :], in0=cs[:, 1, :], in1=tmp8b[:], op=ALU.subtract)
    # fold the qk fp8 scale into the toeplitz generator
    nc.vector.tensor_scalar(out=cs[:, :, :], in0=cs[:, :, :], scalar1=S_QK,
                            scalar2=None, op0=ALU.mult)

    if do_frev:
        BAS = persist.tile([128, 8, 256], BF16)  # [j-part, jt, ca|sa|cb|sb]
        for jt in range(8):
            nc.sync.dma_start(out=BAS[:, jt, :], in_=bas_d.ap()[128 * jt:128 * (jt + 1), :])
        G1 = persist.tile([128, 8, 64], BF16)
        G2 = persist.tile([128, 8, 64], BF16)
        CSst = persist.tile([128, 8, 128], BF16)  # staged cb|sb per jt (DVE-produced)
        for jt in range(8):
            nc.vector.tensor_copy(CSst[:, jt, :], BAS[:, jt, 128:256])
        for jt in range(8):
            ccol = cs[:, 0, jt:jt + 1]
            scol = cs[:, 1, jt:jt + 1]
            cbj, sbj = CSst[:, jt, 0:64], CSst[:, jt, 64:128]
            tmp = small.tile([128, 64], F32, tag="gtmp", name=f"gtmp_{jt}")
            nc.vector.tensor_scalar(out=tmp[:], in0=sbj, scalar1=scol, scalar2=None, op0=ALU.mult)
            nc.vector.scalar_tensor_tensor(out=G1[:, jt, :], in0=cbj, scalar=ccol,
                                           in1=tmp[:], op0=ALU.mult, op1=ALU.add)
            tmp2 = small.tile([128, 64], F32, tag="gtmp2", name=f"gtmp2_{jt}")
            nc.vector.tensor_scalar(out=tmp2[:], in0=sbj, scalar1=ccol, scalar2=None, op0=ALU.mult)
            nc.vector.scalar_tensor_tensor(out=G2[:, jt, :], in0=cbj, scalar=scol,
                                           in1=tmp2[:], op0=ALU.mult, op1=ALU.subtract)
        ps_f = ps_a.tile([64, 64], F32, tag="ps_a")
        for jt in range(8):
            nc.tensor.matmul(out=ps_f[:], lhsT=BAS[:, jt, 0:64], rhs=G1[:, jt, :],
                             start=(jt == 0), stop=False)
            nc.tensor.matmul(out=ps_f[:], lhsT=BAS[:, jt, 64:128], rhs=G2[:, jt, :],
                             start=False, stop=(jt == 7))
        frev_sb = small.tile([64, 64], BF16, tag="frev")
        nc.vector.tensor_copy(frev_sb[:], ps_f[:])
        nc.sync.dma_start(out=frev_d.ap()[:, :], in_=frev_sb[:])
        # cache the 28 distinct diagonal-block H tiles (dd = i - 4*mb);
        # mb0 consumes dd in [0, 15] first, so load those before dd < 0
        for dd in list(range(0, 16)) + list(range(-12, 0)):
            nc.sync.dma_start(out=Hcache[:, dd + 12, :],
                              in_=bass.AP(tensor=frev_d, offset=1921 - 128 * dd,
                                          ap=[[1, 128], [1, 512]]))

    # ---------------- gamma/beta (q-side scaled by S_QK/N on device) --------
    gb = persist.tile([128, 2], F32)  # g0*S_QK/N, b0*S_QK/N
    nc.vector.tensor_scalar(out=gb[:, 0:1], in0=smv[:, 32:33], scalar1=S_QK / N, scalar2=None, op0=ALU.mult)
    nc.vector.tensor_scalar(out=gb[:, 1:2], in0=smv[:, 34:35], scalar1=S_QK / N, scalar2=None, op0=ALU.mult)

    b1col = None
    b1bc = None
    if b1_d is not None:
        b1col = persist.tile([128, 17], F32)
        nc.sync.dma_start(out=b1col[:], in_=b1_d.ap()[:, :])
        b1bc = persist.tile([128, EXPAND], F32)
        nc.sync.dma_start(out=b1bc[:], in_=b1b_d.ap()[:, :])
    b2bc = None
    if b2_d is not None:
        b2bc = persist.tile([128, D], F32)
        nc.sync.dma_start(out=b2bc[:], in_=b2_d.ap()[:, :])

    # ---------------- norm + xn + transpose ----------------
    mscol = persist.tile([128, NT], F32)
    rcol = persist.tile([128, NT], F32)
    sqscr = persist.tile([128, D], F32)
    for g in range(NT // 4):  # groups of 4
        for k4 in range(4):
            mt = 4 * g + k4
            nc.scalar.activation(sqscr[:], xsb[:, mt, :], AF.Square,
                                 accum_out=mscol[:, mt:mt + 1])
        gsl = slice(4 * g, 4 * g + 4)
        # rs = S_X * norm_scale / sqrt(ms/D + eps)
        nc.vector.tensor_scalar(out=rcol[:, gsl], in0=mscol[:, gsl], scalar1=1.0 / D,
                                scalar2=EPS, op0=ALU.mult, op1=ALU.add)
        nc.scalar.sqrt(rcol[:, gsl], rcol[:, gsl])
        nc.vector.reciprocal(rcol[:, gsl], rcol[:, gsl])
        nc.vector.tensor_scalar(out=rcol[:, gsl], in0=rcol[:, gsl], scalar1=smv[:, 36:37],
                                scalar2=S_X, op0=ALU.mult, op1=ALU.mult)
        for k4 in range(4):
            mt = 4 * g + k4
            xn_t = xnpool.tile([128, D], BF16, tag="xn", name=f"xn_{mt}")
            nc.vector.tensor_scalar(out=xn_t[:], in0=xsb[:, mt, :], scalar1=rcol[:, mt:mt + 1],
                                    scalar2=None, op0=ALU.mult)
            ps_tr = ps_a.tile([128, KD, 128], BF16, tag="ps_a", name=f"ps_tr_{mt}")
            for dt in range(KD):
                nc.tensor.transpose(out=ps_tr[:, dt, :], in_=xn_t[:, 128 * dt:128 * (dt + 1)],
                                    identity=identB[:])
            if mt % 2 == 0:
                nc.scalar.activation(xnT[:, :, 128 * mt:128 * (mt + 1)], ps_tr[:, :, :], AF.Copy)
            else:
                nc.vector.tensor_copy(xnT[:, :, 128 * mt:128 * (mt + 1)], ps_tr[:, :, :])

    if stage <= 0:
        for mt in range(NT):
            o_t = outst.tile([128, D], F32, tag="out", name=f"dbg0_{mt}")
            nc.scalar.activation(o_t[:], xnT[:, mt % KD, 512:1024], AF.Copy)
            nc.sync.dma_start(out=out_d.ap()[128 * mt:128 * (mt + 1), :], in_=o_t[:])
        return

    # ---------------- proj1 (fp8 DoubleRow): baseT -> q/k, uT, v ----------
    # baseT [s, m] first: it unlocks q/k and the attention qk matmuls
    bsl = slice(2 * EXPAND, 2 * EXPAND + SHARED)
    for mb in range(MB):
        msl = slice(512 * mb, 512 * (mb + 1))
        ps = ps_a.tile([128, 512], F32, tag="ps_a")
        for k2 in range(KD // 2):
            nc.tensor.matmul(out=ps[:], lhsT=W1sb[:, 2 * k2:2 * k2 + 2, bsl],
                             rhs=xnT[:, 2 * k2:2 * k2 + 2, msl],
                             start=(k2 == 0), stop=(k2 == KD // 2 - 1), perf_mode=DR)
        silu_evict(baseT[:, msl], ps[:],
                   bias=None if b1col is None else b1col[:, 16:17],
                   name=f"sig_b_{mb}")
        # q/k for this m-block (scale S_QK/N folded into g0/b0)
        nc.vector.tensor_scalar(out=qT[:, msl], in0=baseT[:, msl], scalar1=gb[:, 0:1],
                                scalar2=gb[:, 1:2], op0=ALU.mult, op1=ALU.add)
        nc.vector.tensor_scalar(out=kT[:, msl], in0=baseT[:, msl], scalar1=smv[:, 33:34],
                                scalar2=smv[:, 35:36], op0=ALU.mult, op1=ALU.add)
    # v [n, e] (stationary xnT), then Pool converts bf16 -> fp8 (x S_V);
    # the per-mb o1 accumulation chains consume v tiles in nt order, so
    # attention pipelines behind this loop.
    for nt in range(NT):
        nsl = slice(128 * nt, 128 * (nt + 1))
        for vh in range(2):
            vsl = slice(EXPAND + 512 * vh, EXPAND + 512 * (vh + 1))
            ps = ps_a.tile([128, 512], F32, tag="ps_a")
            for k2 in range(KD // 2):
                nc.tensor.matmul(out=ps[:], lhsT=xnT[:, 2 * k2:2 * k2 + 2, nsl],
                                 rhs=W1sb[:, 2 * k2:2 * k2 + 2, vsl],
                                 start=(k2 == 0), stop=(k2 == KD // 2 - 1), perf_mode=DR)
            vout = vsb[:, nt, 512 * vh:512 * (vh + 1)]
            if b1bc is not None:
                zb = silu_pool.tile([128, 512], F32, tag="zb", name=f"zb_{nt}_{vh}")
                nc.vector.scalar_tensor_tensor(out=zb[:], in0=ps[:], scalar=INV_UV,
                                               in1=b1bc[:, 512 * vh:512 * (vh + 1)],
                                               op0=ALU.mult, op1=ALU.add)
                nc.scalar.activation(vout, zb[:], AF.Silu)
            else:
                nc.scalar.activation(vout, ps[:], AF.Silu, scale=INV_UV)

    # deferred: W2 is only needed from the output projection on
    for et in range(CU):
        nc.sync.dma_start(out=W2sb[:, et, :], in_=w2_d.ap()[128 * et:128 * (et + 1), :])

    if stage <= 1:
        for mt in range(NT):
            o_t = outst.tile([128, D], F32, tag="out", name=f"dbg_{mt}")
            nc.scalar.activation(o_t[:], uT[:, mt % CU, 512:1024], AF.Copy)
            nc.sync.dma_start(out=out_d.ap()[128 * mt:128 * (mt + 1), :], in_=o_t[:])
        return

    # ---------------- attention + output proj, per m-block ----------------
    # outproj for mb is emitted after qk/o1 of mb+1 (software pipelining) so
    # the PE never stalls on the o_sb eviction chain (ACT copy -> Pool mult).
    osb_tiles = [None] * MB

    def emit_outproj(mb):
        for mt4 in range(4):
            mt = 4 * mb + mt4
            mrow = 128 * mt
            ps = ps_a.tile([128, 512], F32, tag="ps_a")
            for e2 in range(CU // 2):
                nc.tensor.matmul(out=ps[:],
                                 lhsT=osb_tiles[mb][:, 2 * e2:2 * e2 + 2, 128 * mt4:128 * (mt4 + 1)],
                                 rhs=W2sb[:, 2 * e2:2 * e2 + 2, :],
                                 start=(e2 == 0), stop=(e2 == CU // 2 - 1), perf_mode=DR)
            o_t = outst.tile([128, D], F32, tag="out", name=f"ot_{mb}_{mt4}")
            nc.vector.scalar_tensor_tensor(out=o_t[:], in0=ps[:], scalar=FSCALE,
                                           in1=xsb[:, mt, :], op0=ALU.mult, op1=ALU.add)
            if b2bc is not None:
                nc.vector.tensor_tensor(out=o_t[:], in0=o_t[:], in1=b2bc[:], op=ALU.add)
            nc.sync.dma_start(out=out_d.ap()[mrow:mrow + 128, :], in_=o_t[:])

    # qk + sqrrelu for ALL m-blocks first: this work only needs baseT, so it
    # fills the PE/DVE while the ACT is busy with the u/v silu evictions.
    # relu and square are split across ACT/DVE/Pool per tile (a DVE op may
    # only read PSUM through one input, so relu^2 needs a bf16 staging tile).
    kernTs = [None] * MB
    for mb in range(MB):
        msl = slice(512 * mb, 512 * (mb + 1))
        kernT = kernp.tile([128, NT, 512], F8, tag="kernT", name=f"kernT_{mb}")
        kernTs[mb] = kernT
        for i in range(NT):
            ps = ps_a.tile([128, 512], F32, tag="ps_a")
            nc.tensor.matmul(out=ps[:], lhsT=kT[:, 128 * i:128 * (i + 1)], rhs=qT[:, msl],
                             start=True, stop=False)
            dd = i - 4 * mb
            nc.tensor.matmul(out=ps[:], lhsT=jmatB[:], rhs=Hcache[:, dd + 12, :],
                             start=False, stop=True)
            # kern = relu(z) * z = 2^14 relu(qk+T)^2 -> fp8
            rl = rlp.tile([128, 512], BF16, tag="rl", name=f"rl_{mb}_{i}")
            if i % 16 < 12:
                nc.scalar.activation(rl[:], ps[:], AF.Relu)
            else:
                nc.vector.tensor_scalar(out=rl[:], in0=ps[:], scalar1=0.0, scalar2=None,
                                        op0=ALU.max)
            if i % 16 < 7:
                nc.vector.tensor_tensor(out=kernT[:, i, :], in0=rl[:], in1=rl[:], op=ALU.mult)
            else:
                nc.gpsimd.tensor_tensor(out=kernT[:, i, :], in0=rl[:], in1=rl[:], op=ALU.mult)

    # uT[c, m] (stationary W1), mb-major so mb0's u tiles finish first
    for mb in range(MB):
        msl = slice(512 * mb, 512 * (mb + 1))
        for cu in range(CU):
            csl = slice(128 * cu, 128 * (cu + 1))
            ps = ps_a.tile([128, 512], F32, tag="ps_a")
            for k2 in range(KD // 2):
                nc.tensor.matmul(out=ps[:], lhsT=W1sb[:, 2 * k2:2 * k2 + 2, csl],
                                 rhs=xnT[:, 2 * k2:2 * k2 + 2, msl],
                                 start=(k2 == 0), stop=(k2 == KD // 2 - 1), perf_mode=DR)
            silu_evict(uT[:, cu, msl], ps[:],
                       bias=None if b1col is None else b1col[:, cu:cu + 1],
                       name=f"sig_u_{cu}_{mb}")

    for mb in range(MB):
        msl = slice(512 * mb, 512 * (mb + 1))
        kernT = kernTs[mb]
        o_sb = osbp.tile([128, CU, 512], F8, tag="o_sb", name=f"osb_{mb}")
        for ep in range(2):
            pso = [ps_o1.tile([128, 512], F32, tag="o1", name=f"o1_{mb}_{ep}_{j}") for j in range(4)]
            for i2 in range(NT // 2):
                for et4 in range(4):
                    et = 4 * ep + et4
                    nc.tensor.matmul(out=pso[et4][:],
                                     lhsT=vsb[:, 2 * i2:2 * i2 + 2, 128 * et:128 * (et + 1)],
                                     rhs=kernT[:, 2 * i2:2 * i2 + 2, :],
                                     start=(i2 == 0), stop=(i2 == NT // 2 - 1), perf_mode=DR)
            for et4 in range(4):
                et = 4 * ep + et4
                # o_sb = (o1_psum * 2^-6) * uT -> fp8 (= 2^8 * o)
                nc.vector.scalar_tensor_tensor(out=o_sb[:, et, :], in0=pso[et4][:],
                                               scalar=OSCALE, in1=uT[:, et, msl],
                                               op0=ALU.mult, op1=ALU.mult)
        osb_tiles[mb] = o_sb
        if mb >= 1:
            emit_outproj(mb - 1)
    emit_outproj(MB - 1)


_BUILD_CACHE = {}


def _get_nc(b1_zero, b2_zero, sim_compat=False, stage=2):
    key = (b1_zero, b2_zero, sim_compat, stage)
    if key not in _BUILD_CACHE:
        _BUILD_CACHE[key] = _build(b1_zero, b2_zero, sim_compat, stage)
    return _BUILD_CACHE[key]


def kernel(x, W1, b1, W2, b2, rope_a, rope_b, gamma, beta, norm_scale):
    global LAST_RESULTS
    x = np.asarray(x, dtype=np.float32)
    f8 = ml_dtypes.float8_e4m3
    b1_zero = not np.any(np.asarray(b1))
    b2_zero = not np.any(np.asarray(b2))
    nc = _get_nc(b1_zero, b2_zero, stage=int(os.environ.get('GAU_STAGE', '2')))

    common = {
        "w1": np.clip(np.asarray(W1, np.float32) * S_W, -240, 240).astype(f8),
        "w2": np.clip(np.asarray(W2, np.float32) * S_W, -240, 240).astype(f8),
        "smalls": _pack_smalls(rope_a, rope_b, gamma, beta, norm_scale),
        "basis": _pack_basis(),
    }
    if not b1_zero:
        b1f = np.asarray(b1, np.float32)
        common["b1t"] = np.ascontiguousarray(b1f.reshape(17, 128).T)
        common["b1bc"] = np.broadcast_to(b1f[EXPAND:2 * EXPAND], (128, EXPAND)).copy()
    if not b2_zero:
        common["b2bc"] = np.broadcast_to(np.asarray(b2, np.float32), (128, D)).copy()

    in_maps = [dict(common, x=np.ascontiguousarray(x[i])) for i in range(B)]
    res = run_bass_kernel_spmd(nc, in_maps, list(range(B)),
                               trace=bool(os.environ.get("GAU_TRACE")))
    LAST_RESULTS = res
    out = np.stack([res.results[i]["out"] for i in range(B)]).astype(np.float32)
    return out


# revision 17
# speedup vs baseline: 1.7042x; 1.0088x over previous
"""GAU (gated attention unit) Bass kernel for Trainium2, 8-core data-parallel.

Contract: kernel(**inputs) takes FULL inputs (x [8,2048,512] f32, W1, b1, W2,
b2, rope_a, rope_b, gamma, beta, norm_scale) and returns the full output
[8,2048,512] f32.  Internally: batch b -> NeuronCore b (all params
replicated), one fused Bass/Tile program per core, no collectives.

Math notes:
  - toeplitz bias T[m,n] = sum_j c_j cos((n-m)w_j) + s_j sin((n-m)w_j)
      c_j = a_j b_j + a_{h+j} b_{h+j},  s_j = a_{h+j} b_j - a_j b_{h+j}
    so T is generated by a length-4095 vector f(d).  We compute f (reversed,
    "frev") on device with two tiny matmuls against input-independent trig
    basis matrices (shipped from host), then add T tiles into the qk PSUM
    with a PE matmul against the anti-identity J: (J @ H)[p,e] = H[127-p,e]
    where H[p,e] = frev[B + p + e] is a plain positive-stride DMA load.
    The 28 distinct diagonal-block H tiles are cached in SBUF.
  - fp8 (e4m3, +-240) DoubleRow matmuls for proj1, kernel@v and the output
    projection.  Static power-of-2 scales keep everything in fp8 range:
      W1,W2 x1024 (host)   xn x16   q-side x128/N   frev x128   v x64
    so  uv_psum = 2^14 uv   qk_psum = 128 (qk+T)   kern = relu(z)z = 2^14 kern
        o1_psum = 2^20 o1   o_sb = 2^8 o (fp8)     out_psum = 2^18 o@W2.
  - relu^2 is one DVE op per tile: kern = (ps max 0) * ps -> fp8.
  - residual is a DVE add at the final PSUM evict (x kept resident in SBUF).
"""

import os
from contextlib import ExitStack

import numpy as np
import ml_dtypes

import concourse.bass as bass
import concourse.bacc as bacc
import concourse.tile as tile
from concourse import mybir
from concourse.bass_utils import run_bass_kernel_spmd

F32 = mybir.dt.float32
BF16 = mybir.dt.bfloat16
F8 = mybir.dt.float8e4
AF = mybir.ActivationFunctionType
ALU = mybir.AluOpType
DR = mybir.MatmulPerfMode.DoubleRow

B, N, D = 8, 2048, 512
EXPAND, SHARED = 1024, 128
PROJ = 2 * EXPAND + SHARED  # 2176
EPS = 1e-6
HALF = N // 2  # 1024 rope freqs

NT = N // 128   # 16 seq tiles
KD = D // 128   # 4 contraction tiles for proj1
CU = EXPAND // 128  # 8 u channel tiles
MB = N // 512   # 4 m-blocks of 512
NDD = NT - 1 + MB * 4 - 3  # 28 distinct diagonal blocks (dd in [-12, 15])

S_W = 1024.0        # host scale on W1 and W2
S_X = 16.0          # xn scale (folded into rcol)
INV_UV = 1.0 / (S_W * S_X)   # 2^-14, unfolds uv psum at silu evict
S_QK = 128.0        # qk psum scale (q-side gamma + frev)
S_V = 1.0           # v written fp8 directly by ACT silu
OSCALE = 2.0 ** -6   # = S_o / (S_QK^2 * S_V) with S_o = 256
FSCALE = 2.0 ** -18  # = 1 / (S_o * S_W)

LAST_RESULTS = None  # test introspection


def _basis_matrices():
    """Input-independent trig bases for the on-device Frev computation.

    Frev[p,r] = f_idx(4095 - 64p - r) = f(2048 - 64p - r), built as
    CA @ G1 + SA @ G2 with G1 = CB*c + SB*s, G2 = CB*s - SB*c computed on
    device from the runtime rope_a/rope_b.
    """
    w = 10000.0 ** (-np.arange(HALF, dtype=np.float64) / HALF)
    p = np.arange(64)
    r = np.arange(64)
    alpha = np.outer(2048 - 64 * p, w)  # [64p, 1024j]
    beta = np.outer(-r, w)              # [64r, 1024j]
    ca = np.cos(alpha).T  # [1024j, 64p]  (lhsT: K=j, M=p)
    sa = np.sin(alpha).T
    cb = np.cos(beta).T   # [1024j, 64r]  (rhs template: K=j, N=r)
    sb = np.sin(beta).T
    bf = ml_dtypes.bfloat16
    return (ca.astype(bf), sa.astype(bf), cb.astype(bf), sb.astype(bf))


def _pack_smalls(rope_a, rope_b, gamma, beta, norm_scale):
    sm = np.zeros((128, 37), np.float32)
    sm[:, 0:16] = np.asarray(rope_a, np.float32).reshape(16, 128).T
    sm[:, 16:32] = np.asarray(rope_b, np.float32).reshape(16, 128).T
    g = np.asarray(gamma, np.float32)
    be = np.asarray(beta, np.float32)
    sm[:, 32] = g[0]
    sm[:, 33] = g[1]
    sm[:, 34] = be[0]
    sm[:, 35] = be[1]
    sm[:, 36] = np.float32(np.asarray(norm_scale).reshape(()))
    return sm


def _pack_basis():
    ca, sa, cb, sb = _basis_matrices()
    return np.concatenate([ca, sa, cb, sb], axis=1)  # [1024, 256] bf16


def _build(b1_zero: bool, b2_zero: bool, sim_compat: bool = False, stage: int = 2):
    nc = bacc.Bacc("TRN2", target_bir_lowering=False, debug=False)

    # ---- I/O ----
    x_d = nc.dram_tensor("x", [N, D], F32, kind="ExternalInput")
    w1_d = nc.dram_tensor("w1", [D, PROJ], F8, kind="ExternalInput")
    w2_d = nc.dram_tensor("w2", [EXPAND, D], F8, kind="ExternalInput")
    # smalls: [128, 37] f32, host-packed: 0:16 a[128t+p], 16:32 b[128t+p],
    # 32 gamma0, 33 gamma1, 34 beta0, 35 beta1 (as [128] cols), 36 norm_scale bcast
    sm_d = nc.dram_tensor("smalls", [128, 37], F32, kind="ExternalInput")
    # basis: [1024, 256] bf16: cols 0:64 ca, 64:128 sa, 128:192 cb, 192:256 sb
    bas_d = nc.dram_tensor("basis", [HALF, 256], BF16, kind="ExternalInput")
    b1_d = None if b1_zero else nc.dram_tensor("b1t", [128, 17], F32, kind="ExternalInput")
    b1b_d = None if b1_zero else nc.dram_tensor("b1bc", [128, EXPAND], F32, kind="ExternalInput")
    b2_d = None if b2_zero else nc.dram_tensor("b2bc", [128, D], F32, kind="ExternalInput")
    out_d = nc.dram_tensor("out", [N, D], F32, kind="ExternalOutput")

    frev_d = nc.dram_tensor("frev_scratch", [64, 64], BF16)  # internal

    with tile.TileContext(nc) as tc:
        with ExitStack() as ctx:
            _emit(ctx, tc, nc, locals(), sim_compat, stage)
    nc.compile()
    return nc


def _emit(ctx, tc, nc, t, sim_compat=False, stage=2):
    x_d, w1_d, w2_d = t["x_d"], t["w1_d"], t["w2_d"]
    sm_d, bas_d = t["sm_d"], t["bas_d"]
    b1_d, b1b_d = t["b1_d"], t["b1b_d"]
    b2_d, out_d, frev_d = t["b2_d"], t["out_d"], t["frev_d"]

    # ---------------- pools ----------------
    persist = ctx.enter_context(tc.tile_pool(name="persist", bufs=1))
    xnpool = ctx.enter_context(tc.tile_pool(name="xnpool", bufs=3))
    small = ctx.enter_context(tc.tile_pool(name="small", bufs=2))
    kernp = ctx.enter_context(tc.tile_pool(name="kernp", bufs=4))
    rlp = ctx.enter_context(tc.tile_pool(name="rlp", bufs=6))
    osbp = ctx.enter_context(tc.tile_pool(name="osbp", bufs=2))
    outst = ctx.enter_context(tc.tile_pool(name="outst", bufs=3))
    ps_a = ctx.enter_context(tc.tile_pool(name="ps_a", bufs=4, space="PSUM"))
    ps_o1 = ctx.enter_context(tc.tile_pool(name="ps_o1", bufs=4, space="PSUM"))

    silu_pool = ctx.enter_context(tc.tile_pool(name="silu_pool", bufs=3))

    def silu_evict(out_ap, ps_ap, bias=None, name="sig"):
        # out = silu(ps * INV_UV + bias)
        if bias is not None:
            nc.scalar.activation(out_ap, ps_ap, AF.Silu, bias=bias, scale=INV_UV)
        else:
            nc.scalar.activation(out_ap, ps_ap, AF.Silu, scale=INV_UV)

    # ---------------- persistent SBUF ----------------
    W1sb = persist.tile([128, KD, PROJ], F8)        # [d-part, kt, proj]
    W2sb = persist.tile([128, CU, D], F8)           # [e-part, et, d]
    xnT = persist.tile([128, KD, N], F8)            # [d-part, kt, m] (x16)
    uT = persist.tile([128, CU, N], BF16)           # [e-part, et, m]
    baseT = persist.tile([128, N], BF16)            # [s-part, m]
    qT = persist.tile([128, N], BF16)
    kT = persist.tile([128, N], BF16)
    vsb = persist.tile([128, NT, EXPAND], F8)       # [n-part, nt, e] (x64)
    xsb = persist.tile([128, NT, D], F32)           # resident x (norm + residual)
    Hcache = persist.tile([128, NDD, 512], BF16)    # toeplitz H per diagonal dd
    identB = persist.tile([128, 128], BF16)
    jmatB = persist.tile([128, 128], BF16)

    # x tiles 0-3 first (they gate the norm -> xnT -> proj1 chain), then W1
    for mt in range(4):
        nc.sync.dma_start(out=xsb[:, mt, :], in_=x_d.ap()[128 * mt:128 * (mt + 1), :])
    for kt in range(KD):
        nc.sync.dma_start(out=W1sb[:, kt, :], in_=w1_d.ap()[128 * kt:128 * (kt + 1), :])
    for mt in range(4, NT):
        nc.sync.dma_start(out=xsb[:, mt, :], in_=x_d.ap()[128 * mt:128 * (mt + 1), :])

    # identity / anti-identity (bf16) for PE transpose + toeplitz row-flip
    nc.gpsimd.memset(identB[:], 0.0)
    nc.gpsimd.affine_select(out=identB[:], in_=identB[:], compare_op=ALU.not_equal,
                            fill=1.0, base=0, pattern=[[-1, 128]], channel_multiplier=1)
    nc.gpsimd.memset(jmatB[:], 0.0)
    nc.gpsimd.affine_select(out=jmatB[:], in_=jmatB[:], compare_op=ALU.not_equal,
                            fill=1.0, base=-127, pattern=[[1, 128]], channel_multiplier=1)

    # ---------------- smalls + rope -> frev (one-time, tiny) ----------------
    do_frev = stage >= 1
    smalls = persist.tile([128, 37], F32)
    nc.sync.dma_start(out=smalls[:], in_=sm_d.ap()[:, :])
    cs = persist.tile([128, 2, 8], F32)  # [:,0,:]=c, [:,1,:]=s
    tmp8a = persist.tile([128, 8], F32)
    tmp8b = persist.tile([128, 8], F32)
    # stage smalls into a DVE-produced copy so later DVE ops have all-DVE deps
    smv = persist.tile([128, 37], F32)
    nc.vector.tensor_copy(smv[:], smalls[:])
    asl, bsl2 = smv[:, 0:16], smv[:, 16:32]
    # c = a1*b1 + a2*b2
    nc.vector.tensor_tensor(out=tmp8a[:], in0=asl[:, 0:8], in1=bsl2[:, 0:8], op=ALU.mult)
    nc.vector.scalar_tensor_tensor(out=cs[:, 0, :], in0=asl[:, 8:16], scalar=1.0,
                                   in1=bsl2[:, 8:16], op0=ALU.mult, op1=ALU.mult)
    nc.vector.tensor_tensor(out=cs[:, 0, :], in0=cs[:, 0, :], in1=tmp8a[:], op=ALU.add)
    # s = a2*b1 - a1*b2
    nc.vector.tensor_tensor(out=tmp8b[:], in0=asl[:, 0:8], in1=bsl2[:, 8:16], op=ALU.mult)
    nc.vector.scalar_tensor_tensor(out=cs[:, 1, :], in0=asl[:, 8:16], scalar=1.0,
                                   in1=bsl2[:, 0:8], op0=ALU.mult, op1=ALU.mult)
    nc.vector.tensor_tensor(out=cs[:, 1, :], in0=cs[:, 1, :], in1=tmp8b[:], op=ALU.subtract)
    # fold the qk fp8 scale into the toeplitz generator
    nc.vector.tensor_scalar(out=cs[:, :, :], in0=cs[:, :, :], scalar1=S_QK,
                            scalar2=None, op0=ALU.mult)

    if do_frev:
        BAS = persist.tile([128, 8, 256], BF16)  # [j-part, jt, ca|sa|cb|sb]
        for jt in range(8):
            nc.sync.dma_start(out=BAS[:, jt, :], in_=bas_d.ap()[128 * jt:128 * (jt + 1), :])
        G1 = persist.tile([128, 8, 64], BF16)
        G2 = persist.tile([128, 8, 64], BF16)
        CSst = persist.tile([128, 8, 128], BF16)  # staged cb|sb per jt (DVE-produced)
        for jt in range(8):
            nc.vector.tensor_copy(CSst[:, jt, :], BAS[:, jt, 128:256])
        for jt in range(8):
            ccol = cs[:, 0, jt:jt + 1]
            scol = cs[:, 1, jt:jt + 1]
            cbj, sbj = CSst[:, jt, 0:64], CSst[:, jt, 64:128]
            tmp = small.tile([128, 64], F32, tag="gtmp", name=f"gtmp_{jt}")
            nc.vector.tensor_scalar(out=tmp[:], in0=sbj, scalar1=scol, scalar2=None, op0=ALU.mult)
            nc.vector.scalar_tensor_tensor(out=G1[:, jt, :], in0=cbj, scalar=ccol,
                                           in1=tmp[:], op0=ALU.mult, op1=ALU.add)
            tmp2 = small.tile([128, 64], F32, tag="gtmp2", name=f"gtmp2_{jt}")
            nc.vector.tensor_scalar(out=tmp2[:], in0=sbj, scalar1=ccol, scalar2=None, op0=ALU.mult)
            nc.vector.scalar_tensor_tensor(out=G2[:, jt, :], in0=cbj, scalar=scol,
                                           in1=tmp2[:], op0=ALU.mult, op1=ALU.subtract)
        ps_f = ps_a.tile([64, 64], F32, tag="ps_a")
        for jt in range(8):
            nc.tensor.matmul(out=ps_f[:], lhsT=BAS[:, jt, 0:64], rhs=G1[:, jt, :],
                             start=(jt == 0), stop=False)
            nc.tensor.matmul(out=ps_f[:], lhsT=BAS[:, jt, 64:128], rhs=G2[:, jt, :],
                             start=False, stop=(jt == 7))
        frev_sb = small.tile([64, 64], BF16, tag="frev")
        nc.vector.tensor_copy(frev_sb[:], ps_f[:])
        nc.sync.dma_start(out=frev_d.ap()[:, :], in_=frev_sb[:])
        # cache the 28 distinct diagonal-block H tiles (dd = i - 4*mb);
        # mb0 consumes dd in [0, 15] first, so load those before dd < 0
        for dd in list(range(0, 16)) + list(range(-12, 0)):
            nc.sync.dma_start(out=Hcache[:, dd + 12, :],
                              in_=bass.AP(tensor=frev_d, offset=1921 - 128 * dd,
                                          ap=[[1, 128], [1, 512]]))

    # ---------------- gamma/beta (q-side scaled by S_QK/N on device) --------
    gb = persist.tile([128, 2], F32)  # g0*S_QK/N, b0*S_QK/N
    nc.vector.tensor_scalar(out=gb[:, 0:1], in0=smv[:, 32:33], scalar1=S_QK / N, scalar2=None, op0=ALU.mult)
    nc.vector.tensor_scalar(out=gb[:, 1:2], in0=smv[:, 34:35], scalar1=S_QK / N, scalar2=None, op0=ALU.mult)

    b1col = None
    b1bc = None
    if b1_d is not None:
        b1col = persist.tile([128, 17], F32)
        nc.sync.dma_start(out=b1col[:], in_=b1_d.ap()[:, :])
        b1bc = persist.tile([128, EXPAND], F32)
        nc.sync.dma_start(out=b1bc[:], in_=b1b_d.ap()[:, :])
    b2bc = None
    if b2_d is not None:
        b2bc = persist.tile([128, D], F32)
        nc.sync.dma_start(out=b2bc[:], in_=b2_d.ap()[:, :])

    # ---------------- norm + xn + transpose ----------------
    mscol = persist.tile([128, NT], F32)
    rcol = persist.tile([128, NT], F32)
    sqscr = persist.tile([128, D], F32)
    for g in range(NT // 4):  # groups of 4
        for k4 in range(4):
            mt = 4 * g + k4
            nc.scalar.activation(sqscr[:], xsb[:, mt, :], AF.Square,
                                 accum_out=mscol[:, mt:mt + 1])
        gsl = slice(4 * g, 4 * g + 4)
        # rs = S_X * norm_scale / sqrt(ms/D + eps)
        nc.vector.tensor_scalar(out=rcol[:, gsl], in0=mscol[:, gsl], scalar1=1.0 / D,
                                scalar2=EPS, op0=ALU.mult, op1=ALU.add)
        nc.scalar.sqrt(rcol[:, gsl], rcol[:, gsl])
        nc.vector.reciprocal(rcol[:, gsl], rcol[:, gsl])
        nc.vector.tensor_scalar(out=rcol[:, gsl], in0=rcol[:, gsl], scalar1=smv[:, 36:37],
                                scalar2=S_X, op0=ALU.mult, op1=ALU.mult)
        for k4 in range(4):
            mt = 4 * g + k4
            xn_t = xnpool.tile([128, D], BF16, tag="xn", name=f"xn_{mt}")
            nc.vector.tensor_scalar(out=xn_t[:], in0=xsb[:, mt, :], scalar1=rcol[:, mt:mt + 1],
                                    scalar2=None, op0=ALU.mult)
            ps_tr = ps_a.tile([128, KD, 128], BF16, tag="ps_a", name=f"ps_tr_{mt}")
            for dt in range(KD):
                nc.tensor.transpose(out=ps_tr[:, dt, :], in_=xn_t[:, 128 * dt:128 * (dt + 1)],
                                    identity=identB[:])
            if mt % 2 == 0:
                nc.scalar.activation(xnT[:, :, 128 * mt:128 * (mt + 1)], ps_tr[:, :, :], AF.Copy)
            else:
                nc.vector.tensor_copy(xnT[:, :, 128 * mt:128 * (mt + 1)], ps_tr[:, :, :])

    if stage <= 0:
        for mt in range(NT):
            o_t = outst.tile([128, D], F32, tag="out", name=f"dbg0_{mt}")
            nc.scalar.activation(o_t[:], xnT[:, mt % KD, 512:1024], AF.Copy)
            nc.sync.dma_start(out=out_d.ap()[128 * mt:128 * (mt + 1), :], in_=o_t[:])
        return

    # ---------------- proj1 (fp8 DoubleRow): baseT -> q/k, uT, v ----------
    # baseT [s, m] first: it unlocks q/k and the attention qk matmuls
    bsl = slice(2 * EXPAND, 2 * EXPAND + SHARED)
    for mb in range(MB):
        msl = slice(512 * mb, 512 * (mb + 1))
        ps = ps_a.tile([128, 512], F32, tag="ps_a")
        for k2 in range(KD // 2):
            nc.tensor.matmul(out=ps[:], lhsT=W1sb[:, 2 * k2:2 * k2 + 2, bsl],
                             rhs=xnT[:, 2 * k2:2 * k2 + 2, msl],
                             start=(k2 == 0), stop=(k2 == KD // 2 - 1), perf_mode=DR)
        silu_evict(baseT[:, msl], ps[:],
                   bias=None if b1col is None else b1col[:, 16:17],
                   name=f"sig_b_{mb}")
        # q/k for this m-block (scale S_QK/N folded into g0/b0)
        nc.vector.tensor_scalar(out=qT[:, msl], in0=baseT[:, msl], scalar1=gb[:, 0:1],
                                scalar2=gb[:, 1:2], op0=ALU.mult, op1=ALU.add)
        nc.vector.tensor_scalar(out=kT[:, msl], in0=baseT[:, msl], scalar1=smv[:, 33:34],
                                scalar2=smv[:, 35:36], op0=ALU.mult, op1=ALU.add)
    # v [n, e] (stationary xnT), then Pool converts bf16 -> fp8 (x S_V);
    # the per-mb o1 accumulation chains consume v tiles in nt order, so
    # attention pipelines behind this loop.
    for nt in range(NT):
        nsl = slice(128 * nt, 128 * (nt + 1))
        for vh in range(2):
            vsl = slice(EXPAND + 512 * vh, EXPAND + 512 * (vh + 1))
            ps = ps_a.tile([128, 512], F32, tag="ps_a")
            for k2 in range(KD // 2):
                nc.tensor.matmul(out=ps[:], lhsT=xnT[:, 2 * k2:2 * k2 + 2, nsl],
                                 rhs=W1sb[:, 2 * k2:2 * k2 + 2, vsl],
                                 start=(k2 == 0), stop=(k2 == KD // 2 - 1), perf_mode=DR)
            vout = vsb[:, nt, 512 * vh:512 * (vh + 1)]
            if b1bc is not None:
                zb = silu_pool.tile([128, 512], F32, tag="zb", name=f"zb_{nt}_{vh}")
                nc.vector.scalar_tensor_tensor(out=zb[:], in0=ps[:], scalar=INV_UV,
                                               in1=b1bc[:, 512 * vh:512 * (vh + 1)],
                                               op0=ALU.mult, op1=ALU.add)
                nc.scalar.activation(vout, zb[:], AF.Silu)
            else:
                nc.scalar.activation(vout, ps[:], AF.Silu, scale=INV_UV)

    # deferred: W2 is only needed from the output projection on
    for et in range(CU):
        nc.sync.dma_start(out=W2sb[:, et, :], in_=w2_d.ap()[128 * et:128 * (et + 1), :])

    if stage <= 1:
        for mt in range(NT):
            o_t = outst.tile([128, D], F32, tag="out", name=f"dbg_{mt}")
            nc.scalar.activation(o_t[:], uT[:, mt % CU, 512:1024], AF.Copy)
            nc.sync.dma_start(out=out_d.ap()[128 * mt:128 * (mt + 1), :], in_=o_t[:])
        return

    # ---------------- attention + output proj, per m-block ----------------
    # outproj for mb is emitted after qk/o1 of mb+1 (software pipelining) so
    # the PE never stalls on the o_sb eviction chain (ACT copy -> Pool mult).
    osb_tiles = [None] * MB

    def emit_outproj(mb):
        for mt4 in range(4):
            mt = 4 * mb + mt4
            mrow = 128 * mt
            ps = ps_a.tile([128, 512], F32, tag="ps_a")
            for e2 in range(CU // 2):
                nc.tensor.matmul(out=ps[:],
                                 lhsT=osb_tiles[mb][:, 2 * e2:2 * e2 + 2, 128 * mt4:128 * (mt4 + 1)],
                                 rhs=W2sb[:, 2 * e2:2 * e2 + 2, :],
                                 start=(e2 == 0), stop=(e2 == CU // 2 - 1), perf_mode=DR)
            o_t = outst.tile([128, D], F32, tag="out", name=f"ot_{mb}_{mt4}")
            nc.vector.scalar_tensor_tensor(out=o_t[:], in0=ps[:], scalar=FSCALE,
                                           in1=xsb[:, mt, :], op0=ALU.mult, op1=ALU.add)
            if b2bc is not None:
                nc.vector.tensor_tensor(out=o_t[:], in0=o_t[:], in1=b2bc[:], op=ALU.add)
            nc.sync.dma_start(out=out_d.ap()[mrow:mrow + 128, :], in_=o_t[:])

    # qk + sqrrelu for ALL m-blocks first: this work only needs baseT, so it
    # fills the PE/DVE while the ACT is busy with the u/v silu evictions.
    # relu and square are split across ACT/DVE/Pool per tile (a DVE op may
    # only read PSUM through one input, so relu^2 needs a bf16 staging tile).
    kernTs = [None] * MB
    for mb in range(MB):
        msl = slice(512 * mb, 512 * (mb + 1))
        kernT = kernp.tile([128, NT, 512], F8, tag="kernT", name=f"kernT_{mb}")
        kernTs[mb] = kernT
        for i in range(NT):
            ps = ps_a.tile([128, 512], F32, tag="ps_a")
            nc.tensor.matmul(out=ps[:], lhsT=kT[:, 128 * i:128 * (i + 1)], rhs=qT[:, msl],
                             start=True, stop=False)
            dd = i - 4 * mb
            nc.tensor.matmul(out=ps[:], lhsT=jmatB[:], rhs=Hcache[:, dd + 12, :],
                             start=False, stop=True)
            # kern = relu(z) * z = 2^14 relu(qk+T)^2 -> fp8
            rl = rlp.tile([128, 512], BF16, tag="rl", name=f"rl_{mb}_{i}")
            if i % 2 == 0:
                nc.scalar.activation(rl[:], ps[:], AF.Relu)
            else:
                nc.vector.tensor_scalar(out=rl[:], in0=ps[:], scalar1=0.0, scalar2=None,
                                        op0=ALU.max)
            if i % 16 < 7:
                nc.vector.tensor_tensor(out=kernT[:, i, :], in0=rl[:], in1=rl[:], op=ALU.mult)
            else:
                nc.gpsimd.tensor_tensor(out=kernT[:, i, :], in0=rl[:], in1=rl[:], op=ALU.mult)

    for mb in range(MB):
        msl = slice(512 * mb, 512 * (mb + 1))
        # u evictions land in this window, where the ACT is otherwise idle
        for cu in range(CU):
            csl = slice(128 * cu, 128 * (cu + 1))
            ps = ps_a.tile([128, 512], F32, tag="ps_a")
            for k2 in range(KD // 2):
                nc.tensor.matmul(out=ps[:], lhsT=W1sb[:, 2 * k2:2 * k2 + 2, csl],
                                 rhs=xnT[:, 2 * k2:2 * k2 + 2, msl],
                                 start=(k2 == 0), stop=(k2 == KD // 2 - 1), perf_mode=DR)
            silu_evict(uT[:, cu, msl], ps[:],
                       bias=None if b1col is None else b1col[:, cu:cu + 1],
                       name=f"sig_u_{cu}_{mb}")
        kernT = kernTs[mb]
        o_sb = osbp.tile([128, CU, 512], F8, tag="o_sb", name=f"osb_{mb}")
        for ep in range(2):
            pso = [ps_o1.tile([128, 512], F32, tag="o1", name=f"o1_{mb}_{ep}_{j}") for j in range(4)]
            for i2 in range(NT // 2):
                for et4 in range(4):
                    et = 4 * ep + et4
                    nc.tensor.matmul(out=pso[et4][:],
                                     lhsT=vsb[:, 2 * i2:2 * i2 + 2, 128 * et:128 * (et + 1)],
                                     rhs=kernT[:, 2 * i2:2 * i2 + 2, :],
                                     start=(i2 == 0), stop=(i2 == NT // 2 - 1), perf_mode=DR)
            for et4 in range(4):
                et = 4 * ep + et4
                # o_sb = (o1_psum * 2^-6) * uT -> fp8 (= 2^8 * o)
                nc.vector.scalar_tensor_tensor(out=o_sb[:, et, :], in0=pso[et4][:],
                                               scalar=OSCALE, in1=uT[:, et, msl],
                                               op0=ALU.mult, op1=ALU.mult)
        osb_tiles[mb] = o_sb
        if mb >= 1:
            emit_outproj(mb - 1)
    emit_outproj(MB - 1)


_BUILD_CACHE = {}


def _get_nc(b1_zero, b2_zero, sim_compat=False, stage=2):
    key = (b1_zero, b2_zero, sim_compat, stage)
    if key not in _BUILD_CACHE:
        _BUILD_CACHE[key] = _build(b1_zero, b2_zero, sim_compat, stage)
    return _BUILD_CACHE[key]


def kernel(x, W1, b1, W2, b2, rope_a, rope_b, gamma, beta, norm_scale):
    global LAST_RESULTS
    x = np.asarray(x, dtype=np.float32)
    f8 = ml_dtypes.float8_e4m3
    b1_zero = not np.any(np.asarray(b1))
    b2_zero = not np.any(np.asarray(b2))
    nc = _get_nc(b1_zero, b2_zero, stage=int(os.environ.get('GAU_STAGE', '2')))

    common = {
        "w1": np.clip(np.asarray(W1, np.float32) * S_W, -240, 240).astype(f8),
        "w2": np.clip(np.asarray(W2, np.float32) * S_W, -240, 240).astype(f8),
        "smalls": _pack_smalls(rope_a, rope_b, gamma, beta, norm_scale),
        "basis": _pack_basis(),
    }
    if not b1_zero:
        b1f = np.asarray(b1, np.float32)
        common["b1t"] = np.ascontiguousarray(b1f.reshape(17, 128).T)
        common["b1bc"] = np.broadcast_to(b1f[EXPAND:2 * EXPAND], (128, EXPAND)).copy()
    if not b2_zero:
        common["b2bc"] = np.broadcast_to(np.asarray(b2, np.float32), (128, D)).copy()

    in_maps = [dict(common, x=np.ascontiguousarray(x[i])) for i in range(B)]
    res = run_bass_kernel_spmd(nc, in_maps, list(range(B)),
                               trace=bool(os.environ.get("GAU_TRACE")))
    LAST_RESULTS = res
    out = np.stack([res.results[i]["out"] for i in range(B)]).astype(np.float32)
    return out
